# revision 37
# baseline (speedup 1.0000x reference)
"""MinLSTM Trainium2 kernel: B=8, S=8192, D=512, H=256, 8 NeuronCores.

Strategy: data-parallel over batch (one sequence per core). Per core:
  yT[3H, S] = Wt.T-chunks @ xT-chunks via PE (float32r, full-rate fp32),
  gates from PSUM on ACT/DVE, linear-space recurrence
  h_t = F*h_{t-1} + (1-F)*G via the DVE tensor_tensor_scan instruction.

The reference's log-space cumlogsumexp scan is mathematically the linear
recurrence h_t = f_t*h_{t-1} + i_t*g(h~_t) with normalized gates
F = sigmoid(-(softplus(-f)-softplus(-i))), I = 1-F, and
g(z) = max(sigmoid(z), z+0.5) (the two branches cross at z=0).
The linear recurrence is contraction-stable (F<1), so fp32 suffices.

Host-side staging (not on the HW critical path): transpose x to [D, S]
per batch so the PE's contraction dim (D) lands on SBUF partitions with
S contiguous, transpose W to [D, 3H], and fold g() into the initial
carry h0 = g(h_prev). Output is produced as [H, S] per core and
transposed back on the host.
"""

import sys

import numpy as np

sys.path.insert(0, "/opt/trn_rl_repo")

B, S, D, H = 8, 8192, 512, 256
S_TILE = 512
N_TILES = S // S_TILE
K_CH = D // 128  # 4 contraction chunks
N_CORES = 8

_cache = {}


BF16_SCAN = False  # bf16 F/mV/out: ~2x faster DVE scan path, ~0.3% extra error


def _build_nc(act_subst=None):
    from contextlib import ExitStack

    import concourse.bacc as bacc
    import concourse.tile as tile
    from concourse import mybir

    f32 = mybir.dt.float32
    f32r = mybir.dt.float32r
    Alu = mybir.AluOpType
    Act = mybir.ActivationFunctionType

    bf16 = mybir.dt.bfloat16
    sdt = bf16 if BF16_SCAN else f32

    nc = bacc.Bacc("TRN2", target_bir_lowering=False)
    xt = nc.dram_tensor("xt", [D, S], f32r, kind="ExternalInput")
    wt = nc.dram_tensor("wt", [D, 3 * H], f32r, kind="ExternalInput")
    h0 = nc.dram_tensor("h0", [H, 1], f32, kind="ExternalInput")
    out = nc.dram_tensor("out", [H, S], sdt, kind="ExternalOutput")

    with tile.TileContext(nc) as tc, ExitStack() as ctx:
        const_pool = ctx.enter_context(tc.tile_pool(name="const", bufs=1))
        xin_pool = ctx.enter_context(tc.tile_pool(name="xin", bufs=4))
        fi_pool = ctx.enter_context(tc.tile_pool(name="fi_ps", bufs=2, space="PSUM"))
        h_pool = ctx.enter_context(tc.tile_pool(name="h_ps", bufs=2, space="PSUM"))
        work = ctx.enter_context(tc.tile_pool(name="work", bufs=3))
        hout_pool = ctx.enter_context(tc.tile_pool(name="hout", bufs=4))

        wt_view = wt.rearrange("(k p) n -> p k n", p=128)
        wt_sb = []
        for k in range(K_CH):
            wtk = const_pool.tile([128, 3 * H], f32r, name=f"wt{k}", tag=f"wt{k}")
            nc.gpsimd.dma_start(out=wtk, in_=wt_view[:, k, :])
            wt_sb.append(wtk)
        # carries hold u = -h (negated state; undone on the host)
        h0_sb = const_pool.tile([128, 2], f32)
        nc.gpsimd.dma_start(out=h0_sb, in_=h0.rearrange("(c p) one -> p (c one)", p=128))
        carry = [h0_sb[:, 0:1], h0_sb[:, 1:2]]

        xt_view = xt.rearrange("(k p) s -> p k s", p=128)

        for t in range(N_TILES):
            sl = slice(t * S_TILE, (t + 1) * S_TILE)
            xt_sb = []
            for k in range(K_CH):
                xtk = xin_pool.tile([128, S_TILE], f32r, name=f"xt{k}", tag=f"xt{k}")
                nc.sync.dma_start(out=xtk, in_=xt_view[:, k, sl])
                xt_sb.append(xtk)

            # Steady state batches gate math across both H-chunks (FD=1024,
            # lower per-instruction overhead). Tile 0 runs it per-phase so
            # the DVE/scan chain starts as soon as phase 0's gates exist.
            batched = t > 0

            # ss = [sf0 | si0 | sf1 | si1], sh/gg = [c0 | c1] along free dim
            ss = work.tile([128, 4 * S_TILE], f32, tag="ss")
            gg = work.tile([128, 2 * S_TILE], f32, tag="gg")
            ssv = ss.rearrange("p (c x s) -> p c x s", c=2, x=2)
            s2 = work.tile([128, 2, S_TILE], f32, tag="s2")
            rr = work.tile([128, 2, S_TILE], f32, tag="rr")
            ff = work.tile([128, 2, S_TILE], sdt, tag="ff")
            mv = work.tile([128, 2 * S_TILE], sdt, tag="mv")
            h_ps = h_pool.tile([128, 2 * S_TILE], f32)
            for c in range(2):
                fi_ps = fi_pool.tile([128, 2 * S_TILE], f32)
                csl = slice(c * S_TILE, (c + 1) * S_TILE)
                for k in range(K_CH):
                    rhs = xt_sb[k]
                    st = dict(start=(k == 0), stop=(k == K_CH - 1))
                    nc.tensor.matmul(
                        fi_ps[:, 0:S_TILE],
                        lhsT=wt_sb[k][:, c * 128 : c * 128 + 128],
                        rhs=rhs, **st)
                    nc.tensor.matmul(
                        fi_ps[:, S_TILE : 2 * S_TILE],
                        lhsT=wt_sb[k][:, H + c * 128 : H + c * 128 + 128],
                        rhs=rhs, **st)
                    nc.tensor.matmul(
                        h_ps[:, csl],
                        lhsT=wt_sb[k][:, 2 * H + c * 128 : 2 * H + c * 128 + 128],
                        rhs=rhs, **st)

                # [sigmoid(f) | sigmoid(i)] in one ACT pass over both banks
                nc.scalar.activation(ss[:, 2 * c * S_TILE : 2 * (c + 1) * S_TILE],
                                     fi_ps, Act.Sigmoid)
                if not batched:
                    rn0 = work.tile([128, S_TILE], f32, tag="rn0")
                    nc.scalar.activation(rn0, h_ps[:, csl], Act.Relu, scale=-1.0)
                    nc.scalar.activation(gg[:, csl], rn0, Act.Sigmoid, scale=-1.0)
                    hc0 = work.tile([128, S_TILE], f32, tag="hc0")
                    nc.scalar.activation(hc0, h_ps[:, csl], Act.Relu)
                    nc.gpsimd.dma_start(out=gg[:, csl], in_=hc0, accum_op=Alu.add)
                    nc.vector.tensor_add(s2[:, c, :], ssv[:, c, 0, :], ssv[:, c, 1, :])
                    nc.vector.reciprocal_approx_fast(out=rr[:, c, :], in_=s2[:, c, :])
                    nc.vector.tensor_mul(ff[:, c, :], ssv[:, c, 0, :], rr[:, c, :])
                    nc.vector.scalar_tensor_tensor(
                        mv[:, csl], in0=ff[:, c, :], scalar=-1.0, in1=gg[:, csl],
                        op0=Alu.add, op1=Alu.mult)

            if batched:
                # G = max(sigmoid(h), h+0.5) == sigmoid(-relu(-h)) + relu(h)
                # (exact: h>=0 -> 0.5+h, h<0 -> sigmoid(h)). Built without
                # touching the DVE: three ACT passes + an add-reduce SWDGE DMA.
                rn = work.tile([128, 2 * S_TILE], f32, tag="rn")
                nc.scalar.activation(rn, h_ps, Act.Relu, scale=-1.0)
                nc.scalar.activation(gg, rn, Act.Sigmoid, scale=-1.0)
                hc = work.tile([128, 2 * S_TILE], f32, tag="hc")
                nc.scalar.activation(hc, h_ps, Act.Relu)
                nc.gpsimd.dma_start(out=gg, in_=hc, accum_op=Alu.add)
                # F = sf/(sf+si), mV = (F-1)*G at FD=1024
                sf = ssv[:, :, 0, :]   # [128, 2, S_TILE]
                si = ssv[:, :, 1, :]
                nc.vector.tensor_add(s2, sf, si)
                nc.vector.reciprocal_approx_fast(out=rr, in_=s2)
                nc.vector.tensor_mul(ff, sf, rr)
                nc.vector.scalar_tensor_tensor(
                    mv, in0=ff.rearrange("p c s -> p (c s)"), scalar=-1.0, in1=gg,
                    op0=Alu.add, op1=Alu.mult)
            # scan runs on u = -h: u_t = F*u_{t-1} + mV_t (negation undone on host)
            for c in range(2):
                csl = slice(c * S_TILE, (c + 1) * S_TILE)
                ho = hout_pool.tile([128, S_TILE], sdt, tag=f"ho{c}")
                nc.vector.tensor_tensor_scan(
                    ho, data0=ff[:, c, :], data1=mv[:, csl], initial=carry[c],
                    op0=Alu.mult, op1=Alu.add)
                carry[c] = ho[:, S_TILE - 1 : S_TILE]
                nc.sync.dma_start(out=out[c * 128 : (c + 1) * 128, sl], in_=ho)

    nc.compile()
    return nc


def get_nc():
    if "nc" not in _cache:
        _cache["nc"] = _build_nc()
    return _cache["nc"]


def _stage_inputs(x, h_prev, W):
    """Host-side sharding/layout prep (not on the HW critical path)."""
    x = np.ascontiguousarray(x, dtype=np.float32)
    W = np.ascontiguousarray(W, dtype=np.float32)
    h_prev = np.ascontiguousarray(h_prev, dtype=np.float32)

    wt = np.ascontiguousarray(W.T)  # [D, 3H]
    # carry is u = -h, so feed -g(h_prev); g(z) = z + 0.5 if z >= 0 else sigmoid(z)
    h0 = -np.where(h_prev >= 0, h_prev + 0.5, 1.0 / (1.0 + np.exp(-h_prev)))
    h0 = h0.astype(np.float32)

    in_maps = []
    for b in range(N_CORES):
        in_maps.append({
            "xt": np.ascontiguousarray(x[b].T),       # [D, S]
            "wt": wt,
            "h0": np.ascontiguousarray(h0[b].reshape(H, 1)),
        })
    return in_maps


def kernel(x, h_prev, W):
    from concourse.bass_utils import run_bass_kernel_spmd

    nc = get_nc()
    in_maps = _stage_inputs(x, h_prev, W)
    res = run_bass_kernel_spmd(nc, in_maps, core_ids=list(range(N_CORES)))
    out = np.empty((B, S, H), dtype=np.float32)
    for b in range(N_CORES):
        # kernel scans u = -h; negate while transposing [H, S] -> [S, H]
        u = np.asarray(res.results[b]["out"], dtype=np.float32)
        np.negative(u.T, out=out[b])
    return out


if __name__ == "__main__":
    rng = np.random.default_rng(0)
    x = rng.standard_normal((B, S, D), dtype=np.float32)
    h_prev = rng.standard_normal((B, H), dtype=np.float32)
    W = (rng.standard_normal((3 * H, D), dtype=np.float32) / np.sqrt(D)).astype(np.float32)
    out = kernel(x, h_prev, W)
    print(out.shape, out.dtype, np.abs(out).mean())


# revision 38
# speedup vs baseline: 1.0435x; 1.0435x over previous
"""MinLSTM Trainium2 kernel: B=8, S=8192, D=512, H=256, 8 NeuronCores.

Strategy: data-parallel over batch (one sequence per core). Per core:
  yT[3H, S] = Wt.T-chunks @ xT-chunks via PE (float32r, full-rate fp32),
  gates from PSUM on ACT/DVE, linear-space recurrence
  h_t = F*h_{t-1} + (1-F)*G via the DVE tensor_tensor_scan instruction.

The reference's log-space cumlogsumexp scan is mathematically the linear
recurrence h_t = f_t*h_{t-1} + i_t*g(h~_t) with normalized gates
F = sigmoid(-(softplus(-f)-softplus(-i))), I = 1-F, and
g(z) = max(sigmoid(z), z+0.5) (the two branches cross at z=0).
The linear recurrence is contraction-stable (F<1), so fp32 suffices.

Host-side staging (not on the HW critical path): transpose x to [D, S]
per batch so the PE's contraction dim (D) lands on SBUF partitions with
S contiguous, transpose W to [D, 3H], and fold g() into the initial
carry h0 = g(h_prev). Output is produced as [H, S] per core and
transposed back on the host.
"""

import sys

import numpy as np

sys.path.insert(0, "/opt/trn_rl_repo")

B, S, D, H = 8, 8192, 512, 256
S_TILE = 512
N_TILES = S // S_TILE
K_CH = D // 128  # 4 contraction chunks
N_CORES = 8

_cache = {}


BF16_SCAN = False  # bf16 F/mV/out: ~2x faster DVE scan path, ~0.3% extra error


def _build_nc(act_subst=None):
    from contextlib import ExitStack

    import concourse.bacc as bacc
    import concourse.tile as tile
    from concourse import mybir

    f32 = mybir.dt.float32
    f32r = mybir.dt.float32r
    Alu = mybir.AluOpType
    Act = mybir.ActivationFunctionType

    bf16 = mybir.dt.bfloat16
    sdt = bf16 if BF16_SCAN else f32

    nc = bacc.Bacc("TRN2", target_bir_lowering=False)
    xt = nc.dram_tensor("xt", [D, S], f32r, kind="ExternalInput")
    wt = nc.dram_tensor("wt", [D, 3 * H], f32r, kind="ExternalInput")
    h0 = nc.dram_tensor("h0", [H, 1], f32, kind="ExternalInput")
    out = nc.dram_tensor("out", [H, S], sdt, kind="ExternalOutput")

    with tile.TileContext(nc) as tc, ExitStack() as ctx:
        const_pool = ctx.enter_context(tc.tile_pool(name="const", bufs=1))
        xin_pool = ctx.enter_context(tc.tile_pool(name="xin", bufs=4))
        fi_pool = ctx.enter_context(tc.tile_pool(name="fi_ps", bufs=2, space="PSUM"))
        h_pool = ctx.enter_context(tc.tile_pool(name="h_ps", bufs=2, space="PSUM"))
        work = ctx.enter_context(tc.tile_pool(name="work", bufs=3))
        hout_pool = ctx.enter_context(tc.tile_pool(name="hout", bufs=4))

        wt_view = wt.rearrange("(k p) n -> p k n", p=128)
        wt_sb = []
        for k in range(K_CH):
            wtk = const_pool.tile([128, 3 * H], f32r, name=f"wt{k}", tag=f"wt{k}")
            nc.gpsimd.dma_start(out=wtk, in_=wt_view[:, k, :])
            wt_sb.append(wtk)
        # carries hold u = -h (negated state; undone on the host)
        h0_sb = const_pool.tile([128, 2], f32)
        nc.gpsimd.dma_start(out=h0_sb, in_=h0.rearrange("(c p) one -> p (c one)", p=128))
        carry = [h0_sb[:, 0:1], h0_sb[:, 1:2]]

        xt_view = xt.rearrange("(k p) s -> p k s", p=128)

        for t in range(N_TILES):
            sl = slice(t * S_TILE, (t + 1) * S_TILE)
            xt_sb = []
            for k in range(K_CH):
                xtk = xin_pool.tile([128, S_TILE], f32r, name=f"xt{k}", tag=f"xt{k}")
                nc.sync.dma_start(out=xtk, in_=xt_view[:, k, sl])
                xt_sb.append(xtk)

            # Steady state batches gate math across both H-chunks (FD=1024,
            # lower per-instruction overhead). Tile 0 runs it per-phase so
            # the DVE/scan chain starts as soon as phase 0's gates exist.
            batched = t > 0

            # ss = [sf0 | si0 | sf1 | si1], sh/gg = [c0 | c1] along free dim
            ss = work.tile([128, 4 * S_TILE], f32, tag="ss")
            gg = work.tile([128, 2 * S_TILE], f32, tag="gg")
            ssv = ss.rearrange("p (c x s) -> p c x s", c=2, x=2)
            s2 = work.tile([128, 2, S_TILE], f32, tag="s2")
            rr = work.tile([128, 2, S_TILE], f32, tag="rr")
            ff = work.tile([128, 2, S_TILE], sdt, tag="ff")
            mv = work.tile([128, 2 * S_TILE], sdt, tag="mv")
            h_ps = h_pool.tile([128, 2 * S_TILE], f32)
            for c in range(2):
                fi_ps = fi_pool.tile([128, 2 * S_TILE], f32)
                csl = slice(c * S_TILE, (c + 1) * S_TILE)
                for k in range(K_CH):
                    rhs = xt_sb[k]
                    st = dict(start=(k == 0), stop=(k == K_CH - 1))
                    nc.tensor.matmul(
                        fi_ps[:, 0:S_TILE],
                        lhsT=wt_sb[k][:, c * 128 : c * 128 + 128],
                        rhs=rhs, **st)
                    nc.tensor.matmul(
                        fi_ps[:, S_TILE : 2 * S_TILE],
                        lhsT=wt_sb[k][:, H + c * 128 : H + c * 128 + 128],
                        rhs=rhs, **st)
                    nc.tensor.matmul(
                        h_ps[:, csl],
                        lhsT=wt_sb[k][:, 2 * H + c * 128 : 2 * H + c * 128 + 128],
                        rhs=rhs, **st)

                # [sigmoid(f) | sigmoid(i)] in one ACT pass over both banks
                nc.scalar.activation(ss[:, 2 * c * S_TILE : 2 * (c + 1) * S_TILE],
                                     fi_ps, Act.Sigmoid)
                if not batched:
                    sh = work.tile([128, S_TILE], f32, tag="sh")
                    nc.scalar.activation(sh, h_ps[:, csl], Act.Sigmoid)
                    nc.vector.scalar_tensor_tensor(
                        gg[:, csl], in0=h_ps[:, csl], scalar=0.5, in1=sh,
                        op0=Alu.add, op1=Alu.max)
                    nc.vector.tensor_add(s2[:, c, :], ssv[:, c, 0, :], ssv[:, c, 1, :])
                    nc.vector.reciprocal_approx_fast(out=rr[:, c, :], in_=s2[:, c, :])
                    nc.vector.tensor_mul(ff[:, c, :], ssv[:, c, 0, :], rr[:, c, :])
                    nc.vector.scalar_tensor_tensor(
                        mv[:, csl], in0=ff[:, c, :], scalar=-1.0, in1=gg[:, csl],
                        op0=Alu.add, op1=Alu.mult)

            if batched:
                # G = max(sigmoid(h), h+0.5) == sigmoid(-relu(-h)) + relu(h)
                # (exact: h>=0 -> 0.5+h, h<0 -> sigmoid(h)). Built without
                # touching the DVE: three ACT passes + an add-reduce SWDGE DMA.
                rn = work.tile([128, 2 * S_TILE], f32, tag="rn")
                nc.scalar.activation(rn, h_ps, Act.Relu, scale=-1.0)
                nc.scalar.activation(gg, rn, Act.Sigmoid, scale=-1.0)
                hc = work.tile([128, 2 * S_TILE], f32, tag="hc")
                nc.scalar.activation(hc, h_ps, Act.Relu)
                nc.gpsimd.dma_start(out=gg, in_=hc, accum_op=Alu.add)
                # F = sf/(sf+si), mV = (F-1)*G at FD=1024
                sf = ssv[:, :, 0, :]   # [128, 2, S_TILE]
                si = ssv[:, :, 1, :]
                nc.vector.tensor_add(s2, sf, si)
                nc.vector.reciprocal_approx_fast(out=rr, in_=s2)
                nc.vector.tensor_mul(ff, sf, rr)
                nc.vector.scalar_tensor_tensor(
                    mv, in0=ff.rearrange("p c s -> p (c s)"), scalar=-1.0, in1=gg,
                    op0=Alu.add, op1=Alu.mult)
            # scan runs on u = -h: u_t = F*u_{t-1} + mV_t (negation undone on host)
            for c in range(2):
                csl = slice(c * S_TILE, (c + 1) * S_TILE)
                ho = hout_pool.tile([128, S_TILE], sdt, tag=f"ho{c}")
                nc.vector.tensor_tensor_scan(
                    ho, data0=ff[:, c, :], data1=mv[:, csl], initial=carry[c],
                    op0=Alu.mult, op1=Alu.add)
                carry[c] = ho[:, S_TILE - 1 : S_TILE]
                nc.sync.dma_start(out=out[c * 128 : (c + 1) * 128, sl], in_=ho)

    nc.compile()
    return nc


def get_nc():
    if "nc" not in _cache:
        _cache["nc"] = _build_nc()
    return _cache["nc"]


def _stage_inputs(x, h_prev, W):
    """Host-side sharding/layout prep (not on the HW critical path)."""
    x = np.ascontiguousarray(x, dtype=np.float32)
    W = np.ascontiguousarray(W, dtype=np.float32)
    h_prev = np.ascontiguousarray(h_prev, dtype=np.float32)

    wt = np.ascontiguousarray(W.T)  # [D, 3H]
    # carry is u = -h, so feed -g(h_prev); g(z) = z + 0.5 if z >= 0 else sigmoid(z)
    h0 = -np.where(h_prev >= 0, h_prev + 0.5, 1.0 / (1.0 + np.exp(-h_prev)))
    h0 = h0.astype(np.float32)

    in_maps = []
    for b in range(N_CORES):
        in_maps.append({
            "xt": np.ascontiguousarray(x[b].T),       # [D, S]
            "wt": wt,
            "h0": np.ascontiguousarray(h0[b].reshape(H, 1)),
        })
    return in_maps


def kernel(x, h_prev, W):
    from concourse.bass_utils import run_bass_kernel_spmd

    nc = get_nc()
    in_maps = _stage_inputs(x, h_prev, W)
    res = run_bass_kernel_spmd(nc, in_maps, core_ids=list(range(N_CORES)))
    out = np.empty((B, S, H), dtype=np.float32)
    for b in range(N_CORES):
        # kernel scans u = -h; negate while transposing [H, S] -> [S, H]
        u = np.asarray(res.results[b]["out"], dtype=np.float32)
        np.negative(u.T, out=out[b])
    return out


if __name__ == "__main__":
    rng = np.random.default_rng(0)
    x = rng.standard_normal((B, S, D), dtype=np.float32)
    h_prev = rng.standard_normal((B, H), dtype=np.float32)
    W = (rng.standard_normal((3 * H, D), dtype=np.float32) / np.sqrt(D)).astype(np.float32)
    out = kernel(x, h_prev, W)
    print(out.shape, out.dtype, np.abs(out).mean())
